# revision 1
# baseline (speedup 1.0000x reference)
"""Bilateral blur (kornia 5x5, L1 color distance squared) on 8 TRN2 cores.

Data-parallel: one 1536x2048x3 fp32 image per NeuronCore. Residual form
  out = clip(ctr + (sum_o w_o * d_o) / den, 0, 1),  d_o = I(p+o) - I(p)
with the pair symmetry d_{-o}(p) = -d_{+o}(p-o): each unordered offset pair's
diff/weight planes are computed once and read at two alignments.

Findings baked in:
  - GpSimd activity contends with DVE via the shared SBUF port pair and slows
    every DVE op 30-90% -> all tensor work stays on the Vector engine.
  - fp16 (10-bit mantissa) gives near-fp32 accuracy for the residual form:
    emulated max abs err ~3.5e-5, max rel ~1.8e-3. The weight w is scaled by
    512 (bias += ln 512) to stay clear of fp16's subnormal floor; the scale
    cancels exactly in resid/den.
  - d must be produced by an fp32 subtract from fp32 pixels (relative-error
    regime); quantizing pixels first turns the error absolute and blows up
    the exponent accuracy of borderline weights.
  - t accumulation in fp16 is fine; 16-bit tensor_tensor runs at 2x.

Per-partition layouts (partition p owns img cols [16p-2, 16p+18)):
  T     (R+4) x (20px x 3ch)  fp32 interleaved
  d,|d| (R+2) x (3ch x 20px)  fp16 planar
  t,w   (R+2) x 20            fp16
  prod/resid  R x (3ch x 16)  fp16 planar
  den   R x 16                fp16    r32  R x 16  fp32
  stage R x 48                fp32 interleaved (px,ch) for DMA out
"""

import numpy as np
from contextlib import ExitStack

import concourse.bass as bass
import concourse.bacc as bacc
import concourse.mybir as mybir
import concourse.tile as tile
from concourse.bass_utils import run_bass_kernel_spmd
from bass_rust import VecI64Pair

F32 = mybir.dt.float32
F16 = mybir.dt.float16

H, W, C = 1536, 2048, 3
NCORES = 8
KS = 5
SIGMA_S = 1.0
SIGMA_R = 0.06
ROWE = 60
TCOL = 20
WSCALE = 512.0


def _constants():
    x = (np.arange(KS, dtype=np.float32) - KS // 2).astype(np.float32)
    g = np.exp(-0.5 * (x / np.float32(SIGMA_S)) ** 2).astype(np.float32)
    g = g / g.sum()
    space = np.outer(g, g).astype(np.float32)
    inv2sr2 = -0.5 / (SIGMA_R * SIGMA_R)
    return space, inv2sr2


SPACE, INV2SR2 = _constants()
A_SQ = float(np.sqrt(-INV2SR2))
S_CENTER = float(SPACE[2, 2])
PAIRS = [(0, 1), (0, 2)] + [(dy, dx) for dy in (1, 2) for dx in (-2, -1, 0, 1, 2)]
# pairs whose spatial weight is small enough that fp16-quantized inputs to the
# subtract only perturb the output at the ~2e-3 relative tail level
SMALL_S = {(0, 2), (1, -2), (1, 2), (2, -2), (2, -1), (2, 0), (2, 1), (2, 2)}
T16_SUBS = False  # True: 3.90ms but elem-rel tail 2.8e-2; False: safer 6.8e-3 tail


def _fview(ap2d, off, dims):
    v = ap2d.copy()
    v.offset = v.offset + off
    pdim = list(v.ap)[0]
    v.ap = VecI64Pair([list(pdim)] + [list(d) for d in dims])
    return v


def _dview(dram_ap, off, dims):
    v = dram_ap.copy()
    v.offset = v.offset + off
    v.ap = VecI64Pair([list(d) for d in dims])
    return v


def _pin_act_table_set():
    """Force every activation onto natural_log_exp_and_others (it holds all of
    Abs/Square/Exp/Ln), instead of walrus ping-ponging between exp_and_others
    and natural_log around each block's Ln (2 table reloads per block).
    Other sets are emptied but keep their positions so act_func_set_id
    indices stay aligned with act_info.json."""
    import concourse.hw_specs as hw_specs
    import concourse.bacc as bacc_mod
    orig = hw_specs.get_activation_tables
    if getattr(bacc_mod.get_activation_tables, "_pinned", False):
        return

    def patched(arch):
        t = dict(orig(arch))
        keep = "natural_log_exp_and_others"
        if keep in t:
            t = {k: (v if k == keep else set()) for k, v in t.items()}
        return t

    patched._pinned = True
    bacc_mod.get_activation_tables = patched


def build_nc(h=H, r=96):
    _pin_act_table_set()
    nb_blocks = h // r
    assert h % r == 0
    rowlen = W * C

    nc = bacc.Bacc("TRN2", target_bir_lowering=False, debug=False)
    img = nc.declare_dram_parameter("images", [h, W, C], F32, isOutput=False)
    out = nc.declare_dram_parameter("out", [h, W, C], F32, isOutput=True)
    img_a = img[:]
    out_a = out[:]

    with tile.TileContext(nc) as tc, ExitStack() as ctx:
        cpool = ctx.enter_context(tc.tile_pool(name="consts", bufs=1))
        tpool = ctx.enter_context(tc.tile_pool(name="input", bufs=2))
        rpool = ctx.enter_context(tc.tile_pool(name="resid", bufs=2))
        dnpool = ctx.enter_context(tc.tile_pool(name="den", bufs=2))
        rcpool = ctx.enter_context(tc.tile_pool(name="recip", bufs=1))
        dpool = ctx.enter_context(tc.tile_pool(name="diff", bufs=4))
        apool = ctx.enter_context(tc.tile_pool(name="absd", bufs=2))
        ttpool = ctx.enter_context(tc.tile_pool(name="tplane", bufs=2))
        wpool = ctx.enter_context(tc.tile_pool(name="wplane", bufs=3))
        ppool = ctx.enter_context(tc.tile_pool(name="prod", bufs=2))
        gpool = ctx.enter_context(tc.tile_pool(name="stage", bufs=2))
        t16pool = ctx.enter_context(tc.tile_pool(name="t16", bufs=2))

        consts = cpool.tile([128, 2 + len(PAIRS)], F32)
        ca = consts[:]
        nc.vector.memset(ca[:, 0:1], -1.0)
        nc.vector.memset(ca[:, 1:2], A_SQ)
        for i, (dy, dx) in enumerate(PAIRS):
            s = float(SPACE[dy + 2, dx + 2])
            nc.vector.memset(ca[:, 2 + i:3 + i], float(np.log(s * WSCALE)))
        neg1 = ca[:, 0:1]
        a_sq = ca[:, 1:2]

        def load_rows(ta, tile_r0, n, img_r0, sgn):
            if sgn < 0:
                for i in range(n):
                    load_rows(ta, tile_r0 + i, 1, img_r0 - i, 1)
                return
            rs = rowlen
            base = img_r0 * rowlen
            nc.sync.dma_start(
                out=_fview(ta[1:127], tile_r0 * ROWE, [[ROWE, n], [1, 60]]),
                in_=_dview(img_a, base + 42, [[48, 126], [rs, n], [1, 60]]),
            )
            nc.sync.dma_start(
                out=_fview(ta[0:1], tile_r0 * ROWE + 6, [[ROWE, n], [1, 54]]),
                in_=_dview(img_a, base + 0, [[0, 1], [rs, n], [1, 54]]),
            )
            for do, so in ((0, 6), (3, 3)):
                nc.sync.dma_start(
                    out=_fview(ta[0:1], tile_r0 * ROWE + do, [[ROWE, n], [1, 3]]),
                    in_=_dview(img_a, base + so, [[0, 1], [rs, n], [1, 3]]),
                )
            nc.sync.dma_start(
                out=_fview(ta[127:128], tile_r0 * ROWE, [[ROWE, n], [1, 54]]),
                in_=_dview(img_a, base + 6090, [[0, 1], [rs, n], [1, 54]]),
            )
            for do, so in ((54, 6138), (57, 6135)):
                nc.sync.dma_start(
                    out=_fview(ta[127:128], tile_r0 * ROWE + do, [[ROWE, n], [1, 3]]),
                    in_=_dview(img_a, base + so, [[0, 1], [rs, n], [1, 3]]),
                )

        for b in range(nb_blocks):
            r0 = b * r
            tin = tpool.tile([128, (r + 4) * ROWE], F32)
            ta = tin[:]
            if nb_blocks == 1:
                load_rows(ta, 2, r, 0, 1)
                load_rows(ta, 0, 2, 2, -1)
                load_rows(ta, r + 2, 2, h - 2, -1)
            elif b == 0:
                load_rows(ta, 2, r + 2, 0, 1)
                load_rows(ta, 0, 2, 2, -1)
            elif b == nb_blocks - 1:
                load_rows(ta, 0, r + 2, r0 - 2, 1)
                load_rows(ta, r + 2, 2, h - 2, -1)
            else:
                load_rows(ta, 0, r + 4, r0 - 2, 1)

            t16 = None
            if T16_SUBS:
                # fp16 planar copy of T for the small-weight pairs' subtracts
                t16 = t16pool.tile([128, (r + 4) * ROWE], F16)
                nc.scalar.activation(
                    _fview(t16[:], 0, [[ROWE, r + 4], [TCOL, 3], [1, 20]]),
                    _fview(ta, 0, [[ROWE, r + 4], [1, 3], [3, 20]]),
                    mybir.ActivationFunctionType.Copy)

            resid = rpool.tile([128, r * 48], F16)
            den = dnpool.tile([128, r * 16], F16)
            ra = resid[:]
            da = den[:]
            first_resid = [True]
            first_den = [True]

            for i, (dy, dx) in enumerate(PAIRS):
                qr0 = -dy
                nqr = r + dy
                qc0 = -max(dx, 0)
                col_lo = qc0 + 2
                col_e = col_lo & ~1
                nqc = 16 + abs(dx) + (col_lo - col_e)
                ri0 = qr0 + 2

                dt_ = dpool.tile([128, (r + 2) * ROWE], F16)
                ad_ = apool.tile([128, (r + 2) * ROWE], F16)
                tt_ = ttpool.tile([128, (r + 2) * TCOL], F16)
                tw_ = wpool.tile([128, (r + 2) * TCOL], F16)
                dv, av, tv, wv = dt_[:], ad_[:], tt_[:], tw_[:]

                d_out = _fview(dv, ri0 * ROWE + col_e, [[ROWE, nqr], [TCOL, 3], [1, nqc]])
                if T16_SUBS and (dy, dx) in SMALL_S:
                    # fp16 2x subtract from the planar fp16 copy
                    nc.vector.tensor_tensor(
                        d_out,
                        _fview(t16[:], (ri0 + dy) * ROWE + col_e + dx,
                               [[ROWE, nqr], [TCOL, 3], [1, nqc]]),
                        _fview(t16[:], ri0 * ROWE + col_e,
                               [[ROWE, nqr], [TCOL, 3], [1, nqc]]),
                        mybir.AluOpType.subtract)
                else:
                    nc.vector.tensor_tensor(
                        d_out,
                        _fview(ta, (ri0 + dy) * ROWE + (col_e + dx) * 3,
                               [[ROWE, nqr], [1, 3], [3, nqc]]),
                        _fview(ta, ri0 * ROWE + col_e * 3,
                               [[ROWE, nqr], [1, 3], [3, nqc]]),
                        mybir.AluOpType.subtract)
                nc.scalar.activation(
                    _fview(av, ri0 * ROWE + col_e, [[ROWE, nqr], [TCOL, 3], [1, nqc]]),
                    d_out, mybir.ActivationFunctionType.Abs)
                tq = _fview(tv, ri0 * TCOL + col_e, [[TCOL, nqr], [1, nqc]])
                nc.vector.tensor_tensor(
                    tq,
                    _fview(av, ri0 * ROWE + 0 * TCOL + col_e, [[ROWE, nqr], [1, nqc]]),
                    _fview(av, ri0 * ROWE + 1 * TCOL + col_e, [[ROWE, nqr], [1, nqc]]),
                    mybir.AluOpType.add)
                nc.vector.tensor_tensor(
                    tq, tq,
                    _fview(av, ri0 * ROWE + 2 * TCOL + col_e, [[ROWE, nqr], [1, nqc]]),
                    mybir.AluOpType.add)
                nc.scalar.activation(tq, tq, mybir.ActivationFunctionType.Square,
                                     scale=a_sq)
                wq = _fview(wv, ri0 * TCOL + col_e, [[TCOL, nqr], [1, nqc]])
                nc.scalar.activation(wq, tq, mybir.ActivationFunctionType.Exp,
                                     bias=ca[:, 2 + i:3 + i], scale=neg1)

                # pi(q) = w(q)*d(q) once on the extended grid, in place over d;
                # both directions' contributions are slices of pi:
                #   resid += pi(p) ; resid -= pi(p-o)   (bit-identical to the
                # per-direction products, ~45% fewer multiply elements)
                for ch in range(3):
                    dchq = _fview(dv, ri0 * ROWE + ch * TCOL + col_e,
                                  [[ROWE, nqr], [1, nqc]])
                    nc.vector.tensor_tensor(
                        dchq, dchq,
                        _fview(wv, ri0 * TCOL + col_e, [[TCOL, nqr], [1, nqc]]),
                        mybir.AluOpType.mult)
                for sg in (1, -1):
                    ri, ci = (2, 2) if sg == 1 else (2 - dy, 2 - dx)
                    pi_sl = _fview(dv, ri * ROWE + ci, [[ROWE, r], [TCOL, 3], [1, 16]])
                    if first_resid[0]:
                        assert sg == 1
                        nc.vector.tensor_copy(
                            _fview(ra, 0, [[48, r], [16, 3], [1, 16]]), pi_sl)
                        first_resid[0] = False
                    else:
                        nc.vector.tensor_tensor(
                            _fview(ra, 0, [[48, r], [16, 3], [1, 16]]),
                            _fview(ra, 0, [[48, r], [16, 3], [1, 16]]),
                            pi_sl,
                            mybir.AluOpType.add if sg == 1 else mybir.AluOpType.subtract)
                    wslice = _fview(wv, ri * TCOL + ci, [[TCOL, r], [1, 16]])
                    if first_den[0]:
                        nc.vector.tensor_scalar_add(da, wslice, S_CENTER * WSCALE)
                        first_den[0] = False
                    else:
                        nc.vector.tensor_tensor(da, da, wslice, mybir.AluOpType.add)

            # 1/den (x WSCALE, cancels): r32 = exp(-ln(den))
            r32 = rcpool.tile([128, r * 16], F32)
            rca = r32[:]
            nc.scalar.activation(rca, da, mybir.ActivationFunctionType.Ln)
            nc.scalar.activation(rca, rca, mybir.ActivationFunctionType.Exp,
                                 scale=neg1)
            stage = gpool.tile([128, r * 48], F32)
            sa = stage[:]
            for ch in range(3):
                nc.vector.tensor_tensor(
                    _fview(sa, ch, [[48, r], [3, 16]]),
                    _fview(ra, ch * 16, [[48, r], [1, 16]]),
                    _fview(rca, 0, [[16, r], [1, 16]]),
                    mybir.AluOpType.mult)
            nc.vector.tensor_tensor(
                sa, sa, _fview(ta, 2 * ROWE + 6, [[ROWE, r], [1, 48]]),
                mybir.AluOpType.add)
            nc.vector.tensor_scalar(sa, sa, 0.0, 1.0,
                                    mybir.AluOpType.max, mybir.AluOpType.min)
            nc.sync.dma_start(
                out=_dview(out_a, r0 * rowlen, [[48, 128], [rowlen, r], [1, 48]]),
                in_=_fview(sa, 0, [[48, r], [1, 48]]),
            )
    nc.finalize()
    return nc


_CACHE = {}


def _get_nc(h=H, r=96):
    key = (h, r)
    if key not in _CACHE:
        _CACHE[key] = build_nc(h, r)
    return _CACHE[key]


TRACE = False
LAST_RESULT = None


def kernel(images: np.ndarray) -> np.ndarray:
    global LAST_RESULT
    assert images.shape == (NCORES, H, W, C), images.shape
    nc = _get_nc()
    in_maps = [{"images": np.ascontiguousarray(images[i], dtype=np.float32)}
               for i in range(NCORES)]
    res = run_bass_kernel_spmd(nc, in_maps, core_ids=list(range(NCORES)),
                               trace=TRACE)
    LAST_RESULT = res
    return np.stack([res.results[i]["out"] for i in range(NCORES)], axis=0)



# revision 6
# speedup vs baseline: 1.9567x; 1.9567x over previous
"""Bilateral blur (kornia 5x5, L1 color distance squared) on 8 TRN2 cores.

Data-parallel: one 1536x2048x3 fp32 image per NeuronCore. Residual form
  out = clip(ctr + (sum_o w_o * d_o) / den, 0, 1),  d_o = I(p+o) - I(p)
with the pair symmetry d_{-o}(p) = -d_{+o}(p-o): each unordered offset pair's
diff/weight planes are computed once and read at two alignments.

Findings baked in:
  - GpSimd activity contends with DVE via the shared SBUF port pair and slows
    every DVE op 30-90% -> all tensor work stays on the Vector engine.
  - fp16 (10-bit mantissa) gives near-fp32 accuracy for the residual form:
    emulated max abs err ~3.5e-5, max rel ~1.8e-3. The weight w is scaled by
    512 (bias += ln 512) to stay clear of fp16's subnormal floor; the scale
    cancels exactly in resid/den.
  - d must be produced by an fp32 subtract from fp32 pixels (relative-error
    regime); quantizing pixels first turns the error absolute and blows up
    the exponent accuracy of borderline weights.
  - t accumulation in fp16 is fine; 16-bit tensor_tensor runs at 2x.

Per-partition layouts (partition p owns img cols [16p-2, 16p+18)):
  T     (R+4) x (20px x 3ch)  fp32 interleaved
  d,|d| (R+2) x (3ch x 20px)  fp16 planar
  t,w   (R+2) x 20            fp16
  prod/resid  R x (3ch x 16)  fp16 planar
  den   R x 16                fp16    r32  R x 16  fp32
  stage R x 48                fp32 interleaved (px,ch) for DMA out
"""

import numpy as np
from contextlib import ExitStack

import concourse.bass as bass
import concourse.bacc as bacc
import concourse.mybir as mybir
import concourse.tile as tile
from concourse.bass_utils import run_bass_kernel_spmd
from bass_rust import VecI64Pair

F32 = mybir.dt.float32
F16 = mybir.dt.float16

H, W, C = 1536, 2048, 3
NCORES = 8
KS = 5
SIGMA_S = 1.0
SIGMA_R = 0.06
ROWE = 60
TCOL = 20
WSCALE = 512.0


def _constants():
    x = (np.arange(KS, dtype=np.float32) - KS // 2).astype(np.float32)
    g = np.exp(-0.5 * (x / np.float32(SIGMA_S)) ** 2).astype(np.float32)
    g = g / g.sum()
    space = np.outer(g, g).astype(np.float32)
    inv2sr2 = -0.5 / (SIGMA_R * SIGMA_R)
    return space, inv2sr2


SPACE, INV2SR2 = _constants()
A_SQ = float(np.sqrt(-INV2SR2))
S_CENTER = float(SPACE[2, 2])
PAIRS = [(0, 1), (0, 2)] + [(dy, dx) for dy in (1, 2) for dx in (-2, -1, 0, 1, 2)]
# All subtracts run on fp16 planar copies of T (2x DVE mode). Odd-dx pairs
# read a second planar copy shifted by one px so both operands stay
# 4B-aligned (misaligned fp16 TT drops to 1x, as slow as the fp32 path).


def _fview(ap2d, off, dims):
    v = ap2d.copy()
    v.offset = v.offset + off
    pdim = list(v.ap)[0]
    v.ap = VecI64Pair([list(pdim)] + [list(d) for d in dims])
    return v


def _dview(dram_ap, off, dims):
    v = dram_ap.copy()
    v.offset = v.offset + off
    v.ap = VecI64Pair([list(d) for d in dims])
    return v


def _pin_act_table_set():
    """Force every activation onto natural_log_exp_and_others (it holds all of
    Abs/Square/Exp/Ln), instead of walrus ping-ponging between exp_and_others
    and natural_log around each block's Ln (2 table reloads per block).
    Other sets are emptied but keep their positions so act_func_set_id
    indices stay aligned with act_info.json."""
    import concourse.hw_specs as hw_specs
    import concourse.bacc as bacc_mod
    orig = hw_specs.get_activation_tables
    if getattr(bacc_mod.get_activation_tables, "_pinned", False):
        return

    def patched(arch):
        t = dict(orig(arch))
        keep = "natural_log_exp_and_others"
        if keep in t:
            t = {k: (v if k == keep else set()) for k, v in t.items()}
        return t

    patched._pinned = True
    bacc_mod.get_activation_tables = patched


def build_nc(h=H, r=64):
    _pin_act_table_set()
    nb_blocks = h // r
    assert h % r == 0
    rowlen = W * C

    nc = bacc.Bacc("TRN2", target_bir_lowering=False, debug=False)
    img = nc.declare_dram_parameter("images", [h, W, C], F32, isOutput=False)
    out = nc.declare_dram_parameter("out", [h, W, C], F32, isOutput=True)
    img_a = img[:]
    out_a = out[:]

    with tile.TileContext(nc) as tc, ExitStack() as ctx:
        cpool = ctx.enter_context(tc.tile_pool(name="consts", bufs=1))
        tpool = ctx.enter_context(tc.tile_pool(name="input", bufs=2))
        rpool = ctx.enter_context(tc.tile_pool(name="resid", bufs=2))
        dnpool = ctx.enter_context(tc.tile_pool(name="den", bufs=2))
        rcpool = ctx.enter_context(tc.tile_pool(name="recip", bufs=1))
        dpool = ctx.enter_context(tc.tile_pool(name="diff", bufs=4))
        apool = ctx.enter_context(tc.tile_pool(name="absd", bufs=2))
        ttpool = ctx.enter_context(tc.tile_pool(name="tplane", bufs=2))
        wpool = ctx.enter_context(tc.tile_pool(name="wplane", bufs=3))
        ppool = ctx.enter_context(tc.tile_pool(name="prod", bufs=2))
        gpool = ctx.enter_context(tc.tile_pool(name="stage", bufs=2))
        t16pool = ctx.enter_context(tc.tile_pool(name="t16", bufs=2))
        t16opool = ctx.enter_context(tc.tile_pool(name="t16o", bufs=2))

        consts = cpool.tile([128, 2 + len(PAIRS)], F32)
        ca = consts[:]
        nc.vector.memset(ca[:, 0:1], -1.0)
        nc.vector.memset(ca[:, 1:2], A_SQ)
        for i, (dy, dx) in enumerate(PAIRS):
            s = float(SPACE[dy + 2, dx + 2])
            nc.vector.memset(ca[:, 2 + i:3 + i], float(np.log(s * WSCALE)))
        neg1 = ca[:, 0:1]
        a_sq = ca[:, 1:2]

        def load_rows(ta, tile_r0, n, img_r0, sgn):
            if sgn < 0:
                for i in range(n):
                    load_rows(ta, tile_r0 + i, 1, img_r0 - i, 1)
                return
            rs = rowlen
            base = img_r0 * rowlen
            nc.sync.dma_start(
                out=_fview(ta[1:127], tile_r0 * ROWE, [[ROWE, n], [1, 60]]),
                in_=_dview(img_a, base + 42, [[48, 126], [rs, n], [1, 60]]),
            )
            nc.sync.dma_start(
                out=_fview(ta[0:1], tile_r0 * ROWE + 6, [[ROWE, n], [1, 54]]),
                in_=_dview(img_a, base + 0, [[0, 1], [rs, n], [1, 54]]),
            )
            for do, so in ((0, 6), (3, 3)):
                nc.sync.dma_start(
                    out=_fview(ta[0:1], tile_r0 * ROWE + do, [[ROWE, n], [1, 3]]),
                    in_=_dview(img_a, base + so, [[0, 1], [rs, n], [1, 3]]),
                )
            nc.sync.dma_start(
                out=_fview(ta[127:128], tile_r0 * ROWE, [[ROWE, n], [1, 54]]),
                in_=_dview(img_a, base + 6090, [[0, 1], [rs, n], [1, 54]]),
            )
            for do, so in ((54, 6138), (57, 6135)):
                nc.sync.dma_start(
                    out=_fview(ta[127:128], tile_r0 * ROWE + do, [[ROWE, n], [1, 3]]),
                    in_=_dview(img_a, base + so, [[0, 1], [rs, n], [1, 3]]),
                )

        for b in range(nb_blocks):
            r0 = b * r
            tin = tpool.tile([128, (r + 4) * ROWE], F32)
            ta = tin[:]
            if nb_blocks == 1:
                load_rows(ta, 2, r, 0, 1)
                load_rows(ta, 0, 2, 2, -1)
                load_rows(ta, r + 2, 2, h - 2, -1)
            elif b == 0:
                load_rows(ta, 2, r + 2, 0, 1)
                load_rows(ta, 0, 2, 2, -1)
            elif b == nb_blocks - 1:
                load_rows(ta, 0, r + 2, r0 - 2, 1)
                load_rows(ta, r + 2, 2, h - 2, -1)
            else:
                load_rows(ta, 0, r + 4, r0 - 2, 1)

            # fp16 planar copies of T: t16 at even base, t16o shifted one px
            # left so odd-dx subtracts read 4B-aligned operands.
            t16 = t16pool.tile([128, (r + 4) * ROWE], F16)
            nc.scalar.activation(
                _fview(t16[:], 0, [[ROWE, r + 4], [TCOL, 3], [1, 20]]),
                _fview(ta, 0, [[ROWE, r + 4], [1, 3], [3, 20]]),
                mybir.ActivationFunctionType.Copy)
            t16o = t16opool.tile([128, (r + 4) * ROWE], F16)
            nc.scalar.activation(
                _fview(t16o[:], 0, [[ROWE, r + 4], [TCOL, 3], [1, 18]]),
                _fview(ta, 3, [[ROWE, r + 4], [1, 3], [3, 18]]),
                mybir.ActivationFunctionType.Copy)

            resid = rpool.tile([128, r * 48], F16)
            den = dnpool.tile([128, r * 16], F16)
            ra = resid[:]
            da = den[:]
            first_resid = [True]
            first_den = [True]

            for i, (dy, dx) in enumerate(PAIRS):
                qr0 = -dy
                nqr = r + dy
                qc0 = -max(dx, 0)
                col_lo = qc0 + 2
                col_e = col_lo & ~1
                nqc = 16 + abs(dx) + (col_lo - col_e)
                ri0 = qr0 + 2

                dt_ = dpool.tile([128, (r + 2) * ROWE], F16)
                ad_ = apool.tile([128, (r + 2) * ROWE], F16)
                tt_ = ttpool.tile([128, (r + 2) * TCOL], F16)
                tw_ = wpool.tile([128, (r + 2) * TCOL], F16)
                dv, av, tv, wv = dt_[:], ad_[:], tt_[:], tw_[:]

                d_out = _fview(dv, ri0 * ROWE + col_e, [[ROWE, nqr], [TCOL, 3], [1, nqc]])
                if dx % 2 == 0:
                    shifted = _fview(t16[:], (ri0 + dy) * ROWE + col_e + dx,
                                     [[ROWE, nqr], [TCOL, 3], [1, nqc]])
                else:
                    shifted = _fview(t16o[:], (ri0 + dy) * ROWE + col_e + dx - 1,
                                     [[ROWE, nqr], [TCOL, 3], [1, nqc]])
                nc.vector.tensor_tensor(
                    d_out,
                    shifted,
                    _fview(t16[:], ri0 * ROWE + col_e,
                           [[ROWE, nqr], [TCOL, 3], [1, nqc]]),
                    mybir.AluOpType.subtract)
                nc.scalar.activation(
                    _fview(av, ri0 * ROWE + col_e, [[ROWE, nqr], [TCOL, 3], [1, nqc]]),
                    d_out, mybir.ActivationFunctionType.Abs)
                tq = _fview(tv, ri0 * TCOL + col_e, [[TCOL, nqr], [1, nqc]])
                nc.vector.tensor_tensor(
                    tq,
                    _fview(av, ri0 * ROWE + 0 * TCOL + col_e, [[ROWE, nqr], [1, nqc]]),
                    _fview(av, ri0 * ROWE + 1 * TCOL + col_e, [[ROWE, nqr], [1, nqc]]),
                    mybir.AluOpType.add)
                nc.vector.tensor_tensor(
                    tq, tq,
                    _fview(av, ri0 * ROWE + 2 * TCOL + col_e, [[ROWE, nqr], [1, nqc]]),
                    mybir.AluOpType.add)
                nc.scalar.activation(tq, tq, mybir.ActivationFunctionType.Square,
                                     scale=a_sq)
                wq = _fview(wv, ri0 * TCOL + col_e, [[TCOL, nqr], [1, nqc]])
                nc.scalar.activation(wq, tq, mybir.ActivationFunctionType.Exp,
                                     bias=ca[:, 2 + i:3 + i], scale=neg1)

                # pi(q) = w(q)*d(q) once on the extended grid, in place over d;
                # both directions' contributions are slices of pi:
                #   resid += pi(p) ; resid -= pi(p-o)   (bit-identical to the
                # per-direction products, ~45% fewer multiply elements)
                for ch in range(3):
                    dchq = _fview(dv, ri0 * ROWE + ch * TCOL + col_e,
                                  [[ROWE, nqr], [1, nqc]])
                    nc.vector.tensor_tensor(
                        dchq, dchq,
                        _fview(wv, ri0 * TCOL + col_e, [[TCOL, nqr], [1, nqc]]),
                        mybir.AluOpType.mult)
                for sg in (1, -1):
                    ri, ci = (2, 2) if sg == 1 else (2 - dy, 2 - dx)
                    pi_sl = _fview(dv, ri * ROWE + ci, [[ROWE, r], [TCOL, 3], [1, 16]])
                    if first_resid[0]:
                        assert sg == 1
                        nc.vector.tensor_copy(
                            _fview(ra, 0, [[48, r], [16, 3], [1, 16]]), pi_sl)
                        first_resid[0] = False
                    else:
                        nc.vector.tensor_tensor(
                            _fview(ra, 0, [[48, r], [16, 3], [1, 16]]),
                            _fview(ra, 0, [[48, r], [16, 3], [1, 16]]),
                            pi_sl,
                            mybir.AluOpType.add if sg == 1 else mybir.AluOpType.subtract)
                    wslice = _fview(wv, ri * TCOL + ci, [[TCOL, r], [1, 16]])
                    if first_den[0]:
                        nc.vector.tensor_scalar_add(da, wslice, S_CENTER * WSCALE)
                        first_den[0] = False
                    else:
                        nc.vector.tensor_tensor(da, da, wslice, mybir.AluOpType.add)

            # 1/den (x WSCALE, cancels): r32 = exp(-ln(den))
            r32 = rcpool.tile([128, r * 16], F32)
            rca = r32[:]
            nc.scalar.activation(rca, da, mybir.ActivationFunctionType.Ln)
            nc.scalar.activation(rca, rca, mybir.ActivationFunctionType.Exp,
                                 scale=neg1)
            stage = gpool.tile([128, r * 48], F32)
            sa = stage[:]
            for ch in range(3):
                nc.vector.tensor_tensor(
                    _fview(sa, ch, [[48, r], [3, 16]]),
                    _fview(ra, ch * 16, [[48, r], [1, 16]]),
                    _fview(rca, 0, [[16, r], [1, 16]]),
                    mybir.AluOpType.mult)
            nc.vector.tensor_tensor(
                sa, sa, _fview(ta, 2 * ROWE + 6, [[ROWE, r], [1, 48]]),
                mybir.AluOpType.add)
            nc.vector.tensor_scalar(sa, sa, 0.0, 1.0,
                                    mybir.AluOpType.max, mybir.AluOpType.min)
            nc.sync.dma_start(
                out=_dview(out_a, r0 * rowlen, [[48, 128], [rowlen, r], [1, 48]]),
                in_=_fview(sa, 0, [[48, r], [1, 48]]),
            )
    nc.finalize()
    return nc


_CACHE = {}


def _get_nc(h=H, r=64):
    key = (h, r)
    if key not in _CACHE:
        _CACHE[key] = build_nc(h, r)
    return _CACHE[key]


TRACE = False
LAST_RESULT = None


def kernel(images: np.ndarray) -> np.ndarray:
    global LAST_RESULT
    assert images.shape == (NCORES, H, W, C), images.shape
    nc = _get_nc()
    in_maps = [{"images": np.ascontiguousarray(images[i], dtype=np.float32)}
               for i in range(NCORES)]
    res = run_bass_kernel_spmd(nc, in_maps, core_ids=list(range(NCORES)),
                               trace=TRACE)
    LAST_RESULT = res
    return np.stack([res.results[i]["out"] for i in range(NCORES)], axis=0)



# revision 7
# speedup vs baseline: 1.9973x; 1.0208x over previous
"""Bilateral blur (kornia 5x5, L1 color distance squared) on 8 TRN2 cores.

Data-parallel: one 1536x2048x3 fp32 image per NeuronCore. Residual form
  out = clip(ctr + (sum_o w_o * d_o) / den, 0, 1),  d_o = I(p+o) - I(p)
with pair symmetry d_{-o}(p) = -d_{+o}(p-o): each unordered offset pair's
diff/weight planes are computed once and read at two alignments.

Engine assignment (v7, ~1.96ms from 3.84ms baseline):
  - DVE: subtracts from fp16 planar copies (2x mode; a second one-px-shifted
    planar copy keeps odd-dx pairs 4B-aligned), channel-sum adds, pi=w*d
    multiplies, abs for 4/12 pairs (int16 bitcast AND 0x7FFF at 4x),
    fp16 epilogue mult/add/clip.
  - Scalar: planarize copies, Abs (8/12 pairs), Square, Exp, Ln,
    PSUM->SBUF resid copies, final planar-fp16 -> interleaved-fp32 convert.
  - TensorE: resid/den accumulation as identity matmuls (+I / -I stationary)
    accumulating into per-channel PSUM tiles; every chunk is exactly one
    512-fp32 PSUM bank (start=True clears whole banks - chunks must be
    bank-aligned or earlier accumulation is wiped).
  - Software pipelining: depth-2 lookahead on sub/abs so neither engine
    stalls on the DVE<->Scalar ping-pong; engines execute in program order.
  - GpSimd only for one-time iota (identity build); it contends with DVE
    SBUF ports otherwise.
"""

import numpy as np
from contextlib import ExitStack

import concourse.bass as bass
import concourse.bacc as bacc
import concourse.mybir as mybir
import concourse.tile as tile
from concourse.bass_utils import run_bass_kernel_spmd
from bass_rust import VecI64Pair

F32 = mybir.dt.float32
F16 = mybir.dt.float16

H, W, C = 1536, 2048, 3
NCORES = 8
KS = 5
SIGMA_S = 1.0
SIGMA_R = 0.06
ROWE = 60
TCOL = 20
WSCALE = 512.0


def _constants():
    x = (np.arange(KS, dtype=np.float32) - KS // 2).astype(np.float32)
    g = np.exp(-0.5 * (x / np.float32(SIGMA_S)) ** 2).astype(np.float32)
    g = g / g.sum()
    space = np.outer(g, g).astype(np.float32)
    inv2sr2 = -0.5 / (SIGMA_R * SIGMA_R)
    return space, inv2sr2


SPACE, INV2SR2 = _constants()
A_SQ = float(np.sqrt(-INV2SR2))
S_CENTER = float(SPACE[2, 2])
PAIRS = [(0, 1), (0, 2)] + [(dy, dx) for dy in (1, 2) for dx in (-2, -1, 0, 1, 2)]
# All subtracts run on fp16 planar copies of T (2x DVE mode). Odd-dx pairs
# read a second planar copy shifted by one px so both operands stay
# 4B-aligned (misaligned fp16 TT drops to 1x, as slow as the fp32 path).


def _fview(ap2d, off, dims):
    v = ap2d.copy()
    v.offset = v.offset + off
    pdim = list(v.ap)[0]
    v.ap = VecI64Pair([list(pdim)] + [list(d) for d in dims])
    return v


def _dview(dram_ap, off, dims):
    v = dram_ap.copy()
    v.offset = v.offset + off
    v.ap = VecI64Pair([list(d) for d in dims])
    return v


def _pin_act_table_set():
    """Force every activation onto natural_log_exp_and_others (it holds all of
    Abs/Square/Exp/Ln), instead of walrus ping-ponging between exp_and_others
    and natural_log around each block's Ln (2 table reloads per block).
    Other sets are emptied but keep their positions so act_func_set_id
    indices stay aligned with act_info.json."""
    import concourse.hw_specs as hw_specs
    import concourse.bacc as bacc_mod
    orig = hw_specs.get_activation_tables
    if getattr(bacc_mod.get_activation_tables, "_pinned", False):
        return

    def patched(arch):
        t = dict(orig(arch))
        keep = "natural_log_exp_and_others"
        if keep in t:
            t = {k: (v if k == keep else set()) for k, v in t.items()}
        return t

    patched._pinned = True
    bacc_mod.get_activation_tables = patched


def build_nc(h=H, r=64):
    _pin_act_table_set()
    nb_blocks = h // r
    assert h % r == 0
    rowlen = W * C

    nc = bacc.Bacc("TRN2", target_bir_lowering=False, debug=False)
    img = nc.declare_dram_parameter("images", [h, W, C], F32, isOutput=False)
    out = nc.declare_dram_parameter("out", [h, W, C], F32, isOutput=True)
    img_a = img[:]
    out_a = out[:]

    with tile.TileContext(nc) as tc, ExitStack() as ctx:
        cpool = ctx.enter_context(tc.tile_pool(name="consts", bufs=1))
        tpool = ctx.enter_context(tc.tile_pool(name="input", bufs=2))
        rpool = ctx.enter_context(tc.tile_pool(name="resid", bufs=2))
        rcpool = ctx.enter_context(tc.tile_pool(name="recip", bufs=1))
        dpool = ctx.enter_context(tc.tile_pool(name="diff", bufs=5))
        apool = ctx.enter_context(tc.tile_pool(name="absd", bufs=4))
        ttpool = ctx.enter_context(tc.tile_pool(name="tplane", bufs=3))
        wpool = ctx.enter_context(tc.tile_pool(name="wplane", bufs=3))
        ppool = ctx.enter_context(tc.tile_pool(name="prod", bufs=2))
        gpool = ctx.enter_context(tc.tile_pool(name="stage", bufs=2))
        s16pool = ctx.enter_context(tc.tile_pool(name="s16", bufs=2))
        t16pool = ctx.enter_context(tc.tile_pool(name="t16", bufs=2))
        t16opool = ctx.enter_context(tc.tile_pool(name="t16o", bufs=2))
        psrpool = ctx.enter_context(tc.tile_pool(name="psr", bufs=1, space="PSUM"))
        psdpool = ctx.enter_context(tc.tile_pool(name="psd", bufs=1, space="PSUM"))

        consts = cpool.tile([128, 3 + len(PAIRS)], F32)
        ca = consts[:]
        nc.vector.memset(ca[:, 0:1], -1.0)
        nc.vector.memset(ca[:, 1:2], A_SQ)
        for i, (dy, dx) in enumerate(PAIRS):
            s = float(SPACE[dy + 2, dx + 2])
            nc.vector.memset(ca[:, 2 + i:3 + i], float(np.log(s * WSCALE)))
        nc.vector.memset(ca[:, 2 + len(PAIRS):3 + len(PAIRS)], S_CENTER * WSCALE)
        neg1 = ca[:, 0:1]
        a_sq = ca[:, 1:2]
        den_bias = ca[:, 2 + len(PAIRS):3 + len(PAIRS)]

        # 128x128 fp16 identity: iota(j - p) == 0. Stationary operand for the
        # TensorE identity-accumulate matmuls (resid/den accumulation in PSUM).
        ident_i = cpool.tile([128, 128], mybir.dt.int32)
        nc.gpsimd.iota(ident_i[:], pattern=[[1, 128]], base=0,
                       channel_multiplier=-1)
        ident = cpool.tile([128, 128], F16)
        nc.vector.tensor_scalar(ident[:], ident_i[:], 0, None,
                                mybir.AluOpType.is_equal)
        nident = cpool.tile([128, 128], F16)
        nc.vector.tensor_scalar_mul(nident[:], ident[:], -1.0)

        def load_rows(ta, tile_r0, n, img_r0, sgn):
            if sgn < 0:
                for i in range(n):
                    load_rows(ta, tile_r0 + i, 1, img_r0 - i, 1)
                return
            rs = rowlen
            base = img_r0 * rowlen
            nc.sync.dma_start(
                out=_fview(ta[1:127], tile_r0 * ROWE, [[ROWE, n], [1, 60]]),
                in_=_dview(img_a, base + 42, [[48, 126], [rs, n], [1, 60]]),
            )
            nc.sync.dma_start(
                out=_fview(ta[0:1], tile_r0 * ROWE + 6, [[ROWE, n], [1, 54]]),
                in_=_dview(img_a, base + 0, [[0, 1], [rs, n], [1, 54]]),
            )
            for do, so in ((0, 6), (3, 3)):
                nc.sync.dma_start(
                    out=_fview(ta[0:1], tile_r0 * ROWE + do, [[ROWE, n], [1, 3]]),
                    in_=_dview(img_a, base + so, [[0, 1], [rs, n], [1, 3]]),
                )
            nc.sync.dma_start(
                out=_fview(ta[127:128], tile_r0 * ROWE, [[ROWE, n], [1, 54]]),
                in_=_dview(img_a, base + 6090, [[0, 1], [rs, n], [1, 54]]),
            )
            for do, so in ((54, 6138), (57, 6135)):
                nc.sync.dma_start(
                    out=_fview(ta[127:128], tile_r0 * ROWE + do, [[ROWE, n], [1, 3]]),
                    in_=_dview(img_a, base + so, [[0, 1], [rs, n], [1, 3]]),
                )

        for b in range(nb_blocks):
            r0 = b * r
            tin = tpool.tile([128, (r + 4) * ROWE], F32)
            ta = tin[:]
            if nb_blocks == 1:
                load_rows(ta, 2, r, 0, 1)
                load_rows(ta, 0, 2, 2, -1)
                load_rows(ta, r + 2, 2, h - 2, -1)
            elif b == 0:
                load_rows(ta, 2, r + 2, 0, 1)
                load_rows(ta, 0, 2, 2, -1)
            elif b == nb_blocks - 1:
                load_rows(ta, 0, r + 2, r0 - 2, 1)
                load_rows(ta, r + 2, 2, h - 2, -1)
            else:
                load_rows(ta, 0, r + 4, r0 - 2, 1)

            # fp16 planar copies of T: t16 at even base, t16o shifted one px
            # left so odd-dx subtracts read 4B-aligned operands.
            t16 = t16pool.tile([128, (r + 4) * ROWE], F16)
            nc.scalar.activation(
                _fview(t16[:], 0, [[ROWE, r + 4], [TCOL, 3], [1, 20]]),
                _fview(ta, 0, [[ROWE, r + 4], [1, 3], [3, 20]]),
                mybir.ActivationFunctionType.Copy)
            t16o = t16opool.tile([128, (r + 4) * ROWE], F16)
            nc.scalar.activation(
                _fview(t16o[:], 0, [[ROWE, r + 4], [TCOL, 3], [1, 18]]),
                _fview(t16[:], 1, [[ROWE, r + 4], [TCOL, 3], [1, 18]]),
                mybir.ActivationFunctionType.Copy)

            # Per-channel PSUM resid tiles + den tile; every matmul chunk
            # is exactly one 512-fp32 bank (start=True clears whole banks).
            ps_r0 = psrpool.tile([128, r * 16], F32)
            ps_r1 = psrpool.tile([128, r * 16], F32)
            ps_r2 = psrpool.tile([128, r * 16], F32)
            ps_den = psdpool.tile([128, r * 16], F32)
            prc = [ps_r0[:], ps_r1[:], ps_r2[:]]
            pd = ps_den[:]

            # Software-pipelined pair loop: emit pair i+1's sub/abs ahead of
            # pair i's downstream ops so neither engine stalls on the
            # DVE<->Scalar ping-pong (engines execute in program order).
            geo = []
            for i, (dy, dx) in enumerate(PAIRS):
                qc0 = -max(dx, 0)
                col_lo = qc0 + 2
                col_e = col_lo & ~1
                nqc = 16 + abs(dx) + (col_lo - col_e)
                geo.append((dy, dx, r + dy, col_e, nqc, 2 - dy))
            st = {}

            def do_sub(i):
                dy, dx, nqr, col_e, nqc, ri0 = geo[i]
                dt_ = dpool.tile([128, (r + 2) * ROWE], F16, name="dt_")
                dv = dt_[:]
                d_out = _fview(dv, ri0 * ROWE + col_e,
                               [[ROWE, nqr], [TCOL, 3], [1, nqc]])
                if dx % 2 == 0:
                    shifted = _fview(t16[:], (ri0 + dy) * ROWE + col_e + dx,
                                     [[ROWE, nqr], [TCOL, 3], [1, nqc]])
                else:
                    shifted = _fview(t16o[:], (ri0 + dy) * ROWE + col_e + dx - 1,
                                     [[ROWE, nqr], [TCOL, 3], [1, nqc]])
                nc.vector.tensor_tensor(
                    d_out, shifted,
                    _fview(t16[:], ri0 * ROWE + col_e,
                           [[ROWE, nqr], [TCOL, 3], [1, nqc]]),
                    mybir.AluOpType.subtract)
                st[i] = (dv, d_out)

            def do_abs(i):
                dy, dx, nqr, col_e, nqc, ri0 = geo[i]
                dv, d_out = st[i]
                ad_ = apool.tile([128, (r + 2) * ROWE], F16, name="ad_")
                av = ad_[:]
                av_q = _fview(av, ri0 * ROWE + col_e,
                              [[ROWE, nqr], [TCOL, 3], [1, nqc]])
                if i % 3 == 0:
                    # abs on DVE at 4x: clear fp16 sign bit via int16 AND
                    nc.vector.tensor_scalar(av_q.bitcast(mybir.dt.int16),
                                            d_out.bitcast(mybir.dt.int16),
                                            0x7FFF, None,
                                            mybir.AluOpType.bitwise_and)
                else:
                    nc.scalar.activation(av_q, d_out,
                                         mybir.ActivationFunctionType.Abs)
                st[i] = (dv, av)

            def do_rest(i):
                dy, dx, nqr, col_e, nqc, ri0 = geo[i]
                dv, av = st.pop(i)
                tt_ = ttpool.tile([128, (r + 2) * TCOL], F16, name="tt_")
                tw_ = wpool.tile([128, (r + 2) * TCOL], F16, name="tw_")
                tv, wv = tt_[:], tw_[:]
                tq = _fview(tv, ri0 * TCOL + col_e, [[TCOL, nqr], [1, nqc]])
                nc.vector.tensor_tensor(
                    tq,
                    _fview(av, ri0 * ROWE + 0 * TCOL + col_e, [[ROWE, nqr], [1, nqc]]),
                    _fview(av, ri0 * ROWE + 1 * TCOL + col_e, [[ROWE, nqr], [1, nqc]]),
                    mybir.AluOpType.add)
                nc.vector.tensor_tensor(
                    tq, tq,
                    _fview(av, ri0 * ROWE + 2 * TCOL + col_e, [[ROWE, nqr], [1, nqc]]),
                    mybir.AluOpType.add)
                nc.scalar.activation(tq, tq, mybir.ActivationFunctionType.Square,
                                     scale=a_sq)
                wq = _fview(wv, ri0 * TCOL + col_e, [[TCOL, nqr], [1, nqc]])
                nc.scalar.activation(wq, tq, mybir.ActivationFunctionType.Exp,
                                     bias=ca[:, 2 + i:3 + i], scale=neg1)
                for ch in range(3):
                    dchq = _fview(dv, ri0 * ROWE + ch * TCOL + col_e,
                                  [[ROWE, nqr], [1, nqc]])
                    nc.vector.tensor_tensor(
                        dchq, dchq,
                        _fview(wv, ri0 * TCOL + col_e, [[TCOL, nqr], [1, nqc]]),
                        mybir.AluOpType.mult)
                for sg in (1, -1):
                    ri, ci = (2, 2) if sg == 1 else (2 - dy, 2 - dx)
                    lw = ident[:] if sg == 1 else nident[:]
                    first = (i == 0 and sg == 1)
                    last = (i == len(PAIRS) - 1 and sg == -1)
                    for c0 in range(0, r, 32):
                        for ch in range(3):
                            nc.tensor.matmul(
                                _fview(prc[ch], c0 * 16, [[16, 32], [1, 16]]),
                                lw,
                                _fview(dv, (ri + c0) * ROWE + ch * TCOL + ci,
                                       [[ROWE, 32], [1, 16]]),
                                start=first, stop=last)
                        nc.tensor.matmul(
                            _fview(pd, c0 * 16, [[16, 32], [1, 16]]),
                            ident[:],
                            _fview(wv, (ri + c0) * TCOL + ci,
                                   [[TCOL, 32], [1, 16]]),
                            start=first, stop=last)

            do_sub(0)
            do_abs(0)
            do_sub(1)
            do_abs(1)
            for i in range(len(PAIRS)):
                if i + 2 < len(PAIRS):
                    do_sub(i + 2)
                    do_abs(i + 2)
                do_rest(i)

            # 1/den (x WSCALE, cancels): recip16 = exp(-ln(den + w_ctr))
            r16 = rcpool.tile([128, r * 16], F16)
            rca = r16[:]
            nc.scalar.activation(rca, pd, mybir.ActivationFunctionType.Ln,
                                 bias=den_bias)
            nc.scalar.activation(rca, rca, mybir.ActivationFunctionType.Exp,
                                 scale=neg1)
            resid = rpool.tile([128, r * 48], F16)
            ra = resid[:]
            for ch in range(3):
                nc.scalar.activation(
                    _fview(ra, ch * r * 16, [[1, r * 16]]), prc[ch],
                    mybir.ActivationFunctionType.Copy)
            # fp16 planar stage: resid*recip + ctr, clip, then one scalar
            # transpose-convert to fp32 interleaved for the output DMA.
            s16 = s16pool.tile([128, r * 48], F16)
            sv = s16[:]
            for ch in range(3):
                nc.vector.tensor_tensor(
                    _fview(sv, ch * r * 16, [[16, r], [1, 16]]),
                    _fview(ra, ch * r * 16, [[16, r], [1, 16]]),
                    _fview(rca, 0, [[16, r], [1, 16]]),
                    mybir.AluOpType.mult)
            for ch in range(3):
                nc.vector.tensor_tensor(
                    _fview(sv, ch * r * 16, [[16, r], [1, 16]]),
                    _fview(sv, ch * r * 16, [[16, r], [1, 16]]),
                    _fview(t16[:], 2 * ROWE + ch * TCOL + 2, [[ROWE, r], [1, 16]]),
                    mybir.AluOpType.add)
            nc.vector.tensor_scalar(sv, sv, 0.0, 1.0,
                                    mybir.AluOpType.max, mybir.AluOpType.min)
            stage = gpool.tile([128, r * 48], F32)
            sa = stage[:]
            nc.scalar.activation(
                _fview(sa, 0, [[1, 3], [48, r], [3, 16]]),
                _fview(sv, 0, [[r * 16, 3], [16, r], [1, 16]]),
                mybir.ActivationFunctionType.Copy)
            nc.sync.dma_start(
                out=_dview(out_a, r0 * rowlen, [[48, 128], [rowlen, r], [1, 48]]),
                in_=_fview(sa, 0, [[48, r], [1, 48]]),
            )
    nc.finalize()
    return nc


_CACHE = {}


def _get_nc(h=H, r=64):
    key = (h, r)
    if key not in _CACHE:
        _CACHE[key] = build_nc(h, r)
    return _CACHE[key]


TRACE = False
LAST_RESULT = None


def kernel(images: np.ndarray) -> np.ndarray:
    global LAST_RESULT
    assert images.shape == (NCORES, H, W, C), images.shape
    nc = _get_nc()
    in_maps = [{"images": np.ascontiguousarray(images[i], dtype=np.float32)}
               for i in range(NCORES)]
    res = run_bass_kernel_spmd(nc, in_maps, core_ids=list(range(NCORES)),
                               trace=TRACE)
    LAST_RESULT = res
    return np.stack([res.results[i]["out"] for i in range(NCORES)], axis=0)

